# revision 1
# baseline (speedup 1.0000x reference)
"""GATv2 state encoder on 8 Trainium2 NeuronCores (Bass/Tile).

Sharding: nodes split 8 ways by id (6250/core); each directed edge (plus self
loops) is processed by the core owning its dst. Per core, edges are grouped
into 128-node blocks and 128-edge chunks (chunks never cross blocks; within a
block edges are split by src half for int16 gather indices, each side padded
to KA=KB=8 chunks). The small weights are replicated; per-edge source features
are fetched with batched dma_gather from full node-major tables.

Per conv: dense phase builds tables xl = x@(Wl@Win).T (+folded bias) node-
major in DRAM (stationary-xT matmuls); edge phase per block gathers xl[src],
xr[dst], z = xl+xr, Lrelu, att-weighted per-head reduce -> logits, exp (no
max subtraction needed: logits are O(10) in fp32), payload xl*ex, and a
one-hot slot matmul accumulates [sum ex*xl | sum ex] in PSUM over the block's
chunks; node phase h = relu(num/s (+bias)). conv1 returns h1 shards to the
host, which reassembles and launches conv2 (tables padded 32->64 cols);
conv2 also pool-sums h2 over local nodes on-device. Final tiny [1,32]@Wout.T
runs on host (G=1).
"""
import numpy as np
import ml_dtypes

N = 50000
NC = 8
NSH = N // NC              # 6250
NBLK = (NSH + 127) // 128  # 49
HALF = 25088               # src half split (128-aligned, int16-safe)
KA = 8
KB = 8
KCH = KA + KB
P = 128
NEG = 0.2

_cache = {}


def preprocess(edge_index):
    src = np.concatenate([np.asarray(edge_index[0], np.int64),
                          np.arange(N, dtype=np.int64)])
    dst = np.concatenate([np.asarray(edge_index[1], np.int64),
                          np.arange(N, dtype=np.int64)])
    order = np.argsort(dst, kind='stable')
    src, dst = src[order], dst[order]

    def wrap(ix):  # [n] -> [128, n//16] int16, 16-partition wrapped, 8x repl
        n = ix.shape[0]
        a = np.zeros((16, n // 16), np.int16)
        a[np.arange(n) % 16, np.arange(n) // 16] = ix.astype(np.int16)
        return np.tile(a, (8, 1))

    S1, S2, S3 = KA * P // 16, KB * P // 16, KCH * P // 16
    SB = S1 + S2 + S3
    cores = []
    for c in range(NC):
        lo, hi = c * NSH, (c + 1) * NSH
        m = (dst >= lo) & (dst < hi)
        s, d = src[m], (dst[m] - lo)
        srcs = np.zeros((NBLK, KCH, P), np.int64)
        slot = np.full((NBLK, KCH, P), 255, np.int32)
        dstl = np.zeros((NBLK, KCH, P), np.int64)
        for b in range(NBLK):
            mm = (d >= b * 128) & (d < (b + 1) * 128)
            sb, db = s[mm], d[mm]
            amask = sb < HALF
            for side in range(2):
                ss = sb[amask] if side == 0 else sb[~amask]
                dd = db[amask] if side == 0 else db[~amask]
                k0, kmax = (0, KA) if side == 0 else (KA, KB)
                cnt = ss.shape[0]
                assert cnt <= kmax * P
                for k in range((cnt + P - 1) // P):
                    n = min(P, cnt - k * P)
                    sl = slice(k * P, k * P + n)
                    srcs[b, k0 + k, :n] = ss[sl]
                    dstl[b, k0 + k, :n] = dd[sl]
                    slot[b, k0 + k, :n] = dd[sl] - b * 128

        idx_all = np.zeros((128, NBLK * SB), np.int16)
        for b in range(NBLK):
            o = b * SB
            va = slot[b, :KA].reshape(-1) < 128
            vb = slot[b, KA:].reshape(-1) < 128
            sa = np.where(va, srcs[b, :KA].reshape(-1), 0)
            sbb = np.where(vb, srcs[b, KA:].reshape(-1) - HALF, 0)
            dr = np.where(slot[b].reshape(-1) < 128, dstl[b].reshape(-1), 0)
            idx_all[:, o:o + S1] = wrap(sa)
            idx_all[:, o + S1:o + S1 + S2] = wrap(sbb)
            idx_all[:, o + S1 + S2:o + SB] = wrap(dr)

        # Msel: [NBLK, 128e, KCH, 128n] (contiguous for the per-block DMA)
        msel = np.zeros((NBLK, P, KCH, P), ml_dtypes.bfloat16)
        ar = np.arange(P)
        for b in range(NBLK):
            for k in range(KCH):
                sl = slot[b, k]
                v = sl < 128
                msel[b, ar[v], k, sl[v]] = 1

        pmask = np.zeros((NBLK * P,), np.float32)
        pmask[:NSH] = 1
        cores.append(dict(idx_all=idx_all, msel=msel,
                          pmask=np.ascontiguousarray(
                              pmask.reshape(NBLK, P).T)))
    return cores


def build_conv(CT, CE, H, has_bl, has_br, has_bo, do_pool):
    import concourse.bass as bass
    import concourse.mybir as mybir
    import concourse.tile as tile
    import concourse.bacc as bacc

    CH = CT // H
    CEH = CE // H
    S1, S2, S3 = KA * P // 16, KB * P // 16, KCH * P // 16
    SB = S1 + S2 + S3
    NT = (N + P - 1) // P
    LAST = N - (NT - 1) * P
    NTL = NBLK
    LASTL = NSH - (NTL - 1) * P
    DB = 4                      # dense tiles per macro chunk
    NMAC = (NT + DB - 1) // DB

    nc = bacc.Bacc("TRN2")
    dt = mybir.dt
    f32, bf16, i16 = dt.float32, dt.bfloat16, dt.int16

    d_xT = nc.dram_tensor("xT", [P, NMAC * DB * P], f32, kind="ExternalInput")
    d_xTl = nc.dram_tensor("xTl", [P, NBLK * P], f32, kind="ExternalInput")
    d_WA = nc.dram_tensor("WA", [P, CT], f32, kind="ExternalInput")
    d_WB = nc.dram_tensor("WB", [P, CT], f32, kind="ExternalInput")
    d_attr = nc.dram_tensor("attr", [P, CE], bf16, kind="ExternalInput")
    d_attr2 = nc.dram_tensor("attr2", [P, CE], f32, kind="ExternalInput")
    d_idx = nc.dram_tensor("idx", [P, NBLK * SB], i16, kind="ExternalInput")
    d_msel = nc.dram_tensor("msel", [NBLK, P, KCH, P], bf16,
                            kind="ExternalInput")
    d_bl = nc.dram_tensor("bl", [P, CT], f32, kind="ExternalInput") \
        if has_bl else None
    d_br = nc.dram_tensor("br", [P, CT], f32, kind="ExternalInput") \
        if has_br else None
    d_bo = nc.dram_tensor("bo", [P, CT], f32, kind="ExternalInput") \
        if has_bo else None
    d_pm = nc.dram_tensor("pmask", [P, NBLK], f32, kind="ExternalInput") \
        if do_pool else None

    d_tA = nc.dram_tensor("tabA", [HALF, CE], f32, kind="ExternalOutput")
    d_tB = nc.dram_tensor("tabB", [HALF, CE], f32, kind="ExternalOutput")
    d_tR = nc.dram_tensor("tabR", [NBLK * P, CE], f32, kind="ExternalOutput")
    d_h = nc.dram_tensor("h_out", [NSH, CT], f32, kind="ExternalOutput")
    d_pool = nc.dram_tensor("pool_out", [1, CT], f32, kind="ExternalOutput") \
        if do_pool else None

    with tile.TileContext(nc) as tc:
        with (
            tc.tile_pool(name="const", bufs=1) as constp,
            tc.tile_pool(name="dense_in", bufs=3) as dinp,
            tc.tile_pool(name="dense_out", bufs=3) as doutp,
            tc.tile_pool(name="dpsum", bufs=4, space="PSUM") as dpsum,
        ):
            t_WA = constp.tile([P, CT], f32)
            nc.sync.dma_start(t_WA[:], d_WA[:])
            t_WB = constp.tile([P, CT], f32)
            nc.sync.dma_start(t_WB[:], d_WB[:])
            t_bl = t_br = None
            if has_bl:
                t_bl = constp.tile([P, CT], f32)
                nc.sync.dma_start(t_bl[:], d_bl[:])
            if has_br:
                t_br = constp.tile([P, CT], f32)
                nc.sync.dma_start(t_br[:], d_br[:])

            # full xl table (A/B halves)
            for mc in range(NMAC):
                t_x = dinp.tile([P, DB, P], f32, tag="xin")
                nc.sync.dma_start(
                    t_x[:], d_xT[:, mc * DB * P:(mc + 1) * DB * P])
                t_o = doutp.tile([P, DB, CT], f32, tag="dout")
                for j in range(DB):
                    nt = mc * DB + j
                    if nt >= NT:
                        continue
                    m = P if nt < NT - 1 else LAST
                    ps = dpsum.tile([P, CT], f32, tag="dps")
                    nc.tensor.matmul(ps[0:m, :], lhsT=t_x[:, j, 0:m],
                                     rhs=t_WA[:], start=True, stop=True)
                    if has_bl:
                        nc.vector.tensor_tensor(
                            out=t_o[0:m, j, :], in0=ps[0:m, :],
                            in1=t_bl[0:m, :], op=mybir.AluOpType.add)
                    else:
                        nc.scalar.copy(t_o[0:m, j, :], ps[0:m, :])
                g0 = mc * DB * P
                rows = min(NT * P, (mc + 1) * DB * P) - g0
                rows = min(rows, N - g0)
                # DMA out: dram rows (j*128+p) <- sbuf [p, j]
                nfull = rows // P
                if nfull:
                    if g0 + nfull * P <= HALF:
                        dst = d_tA[g0:g0 + nfull * P, 0:CT]
                    else:
                        dst = d_tB[g0 - HALF:g0 - HALF + nfull * P, 0:CT]
                    nc.sync.dma_start(
                        dst.rearrange("(j p) c -> p j c", p=P),
                        t_o[:, 0:nfull, :])
                rem = rows - nfull * P
                if rem:
                    g1 = g0 + nfull * P
                    dst = d_tB[g1 - HALF:g1 - HALF + rem, 0:CT]
                    nc.sync.dma_start(dst, t_o[0:rem, nfull, :])

            # local xr table
            for mc in range((NTL + DB - 1) // DB):
                t_x = dinp.tile([P, DB, P], f32, tag="xin")
                c0 = mc * DB * P
                cols = min(DB * P, NBLK * P - c0)
                nc.sync.dma_start(t_x[:, 0:cols // P, :],
                                  d_xTl[:, c0:c0 + cols])
                t_o = doutp.tile([P, DB, CT], f32, tag="dout")
                for j in range(cols // P):
                    nt = mc * DB + j
                    m = P if nt < NTL - 1 else LASTL
                    ps = dpsum.tile([P, CT], f32, tag="dps")
                    nc.tensor.matmul(ps[0:m, :], lhsT=t_x[:, j, 0:m],
                                     rhs=t_WB[:], start=True, stop=True)
                    if has_br:
                        nc.vector.tensor_tensor(
                            out=t_o[0:m, j, :], in0=ps[0:m, :],
                            in1=t_br[0:m, :], op=mybir.AluOpType.add)
                    else:
                        nc.scalar.copy(t_o[0:m, j, :], ps[0:m, :])
                nc.sync.dma_start(
                    d_tR[c0:c0 + cols, 0:CT].rearrange("(j p) c -> p j c",
                                                       p=P),
                    t_o[:, 0:cols // P, :])

        with (
            tc.tile_pool(name="gat", bufs=2) as gat,
            tc.tile_pool(name="gsm", bufs=2) as gsm,
            tc.tile_pool(name="epsum", bufs=2, space="PSUM") as epsum,
            tc.tile_pool(name="ppsum", bufs=1, space="PSUM") as ppsum,
            tc.tile_pool(name="const2", bufs=1) as constp2,
        ):
            t_attr = constp2.tile([P, CE], bf16)
            nc.sync.dma_start(t_attr[:], d_attr[:])
            t_attr2 = constp2.tile([P, CE], f32)
            nc.sync.dma_start(t_attr2[:], d_attr2[:])
            t_idx = constp2.tile([P, NBLK * SB], i16)
            nc.sync.dma_start(t_idx[:], d_idx[:])
            t_bo = None
            if has_bo:
                t_bo = constp2.tile([P, CT], f32)
                nc.sync.dma_start(t_bo[:], d_bo[:])
            t_pm = None
            t_pool = None
            if do_pool:
                t_pm = constp2.tile([P, NBLK], f32)
                nc.sync.dma_start(t_pm[:], d_pm[:])
                t_pool = ppsum.tile([1, CT], f32)

            for b in range(NBLK):
                o = b * SB
                t_xl = gat.tile([P, KCH, CE], f32, tag="xl")
                nc.gpsimd.dma_gather(
                    out_ap=t_xl[:, 0:KA, :], in_ap=d_tA[:],
                    idxs_ap=t_idx[:, o:o + S1],
                    num_idxs=KA * P, num_idxs_reg=KA * P, elem_size=CE)
                nc.gpsimd.dma_gather(
                    out_ap=t_xl[:, KA:KCH, :], in_ap=d_tB[:],
                    idxs_ap=t_idx[:, o + S1:o + S1 + S2],
                    num_idxs=KB * P, num_idxs_reg=KB * P, elem_size=CE)
                t_xr = gat.tile([P, KCH, CE], f32, tag="xr")
                half3 = S3 // 2
                nc.gpsimd.dma_gather(
                    out_ap=t_xr[:, 0:KCH // 2, :], in_ap=d_tR[:],
                    idxs_ap=t_idx[:, o + S1 + S2:o + S1 + S2 + half3],
                    num_idxs=KCH * P // 2, num_idxs_reg=KCH * P // 2,
                    elem_size=CE)
                nc.gpsimd.dma_gather(
                    out_ap=t_xr[:, KCH // 2:KCH, :], in_ap=d_tR[:],
                    idxs_ap=t_idx[:, o + S1 + S2 + half3:o + SB],
                    num_idxs=KCH * P // 2, num_idxs_reg=KCH * P // 2,
                    elem_size=CE)
                t_msel = gsm.tile([P, KCH, P], bf16, tag="ms")
                nc.sync.dma_start(t_msel[:], d_msel[b])

                t_z = gat.tile([P, KCH, CE], f32, tag="z")
                nc.vector.tensor_tensor(out=t_z[:], in0=t_xl[:], in1=t_xr[:],
                                        op=mybir.AluOpType.add)
                t_zp = gsm.tile([P, KCH, CE], bf16, tag="zp")
                nc.scalar.activation(t_zp[:], t_z[:],
                                     mybir.ActivationFunctionType.Relu)
                # lrelu(z).att = (0.8 att).relu(z) + (0.2 att).z
                t_am = gsm.tile([P, KCH, 2, CE], bf16, tag="am")
                attb = t_attr[:].unsqueeze(1).to_broadcast([P, KCH, CE])
                nc.vector.tensor_tensor(out=t_am[:, :, 0, :], in0=t_zp[:],
                                        in1=attb, op=mybir.AluOpType.mult)
                att2b = t_attr2[:].unsqueeze(1).to_broadcast([P, KCH, CE])
                nc.vector.tensor_tensor(out=t_am[:, :, 1, :], in0=t_z[:],
                                        in1=att2b, op=mybir.AluOpType.mult)
                t_red = gsm.tile([P, KCH, H], f32, tag="red")
                am_g = t_am[:].rearrange("p k s (h c) -> p k h s c", h=H)
                nc.vector.tensor_reduce(out=t_red[:], in_=am_g,
                                        axis=mybir.AxisListType.XY,
                                        op=mybir.AluOpType.add)
                t_ex = gsm.tile([P, KCH, H], f32, tag="ex")
                nc.scalar.activation(t_ex[:], t_red[:],
                                     mybir.ActivationFunctionType.Exp)
                t_pay = gsm.tile([P, KCH, CE + H], bf16, tag="pay")
                ex_b = t_ex[:].unsqueeze(3).to_broadcast([P, KCH, H, CEH])
                pay4 = t_pay[:, :, 0:CE].rearrange("p k (h c) -> p k h c",
                                                   h=H)
                xl4 = t_xl[:].rearrange("p k (h c) -> p k h c", h=H)
                nc.vector.tensor_tensor(out=pay4, in0=xl4, in1=ex_b,
                                        op=mybir.AluOpType.mult)
                nc.vector.tensor_copy(t_pay[:, :, CE:CE + H], t_ex[:])

                t_seg = epsum.tile([P, CE + H], f32, tag="seg")
                for k in range(KCH):
                    nc.tensor.matmul(t_seg[:], lhsT=t_msel[:, k, :],
                                     rhs=t_pay[:, k, :],
                                     start=(k == 0), stop=(k == KCH - 1))

                t_s = gsm.tile([P, H], f32, tag="s")
                nc.vector.tensor_scalar(out=t_s[:], in0=t_seg[:, CE:CE + H],
                                        scalar1=1e-30, scalar2=None,
                                        op0=mybir.AluOpType.max)
                t_rec = gsm.tile([P, H], f32, tag="rec")
                nc.vector.reciprocal(t_rec[:], t_s[:])
                t_hn = gsm.tile([P, CT], f32, tag="hn")
                rec_b = t_rec[:].unsqueeze(2).to_broadcast([P, H, CH])
                hn3 = t_hn[:].rearrange("p (h c) -> p h c", h=H)
                seg3 = t_seg[:, 0:CE].rearrange("p (h c) -> p h c", h=H)
                nc.vector.tensor_tensor(out=hn3, in0=seg3[:, :, 0:CH],
                                        in1=rec_b, op=mybir.AluOpType.mult)
                if has_bo:
                    t_hb = gsm.tile([P, CT], f32, tag="hb")
                    nc.vector.tensor_tensor(out=t_hb[:], in0=t_hn[:],
                                            in1=t_bo[:],
                                            op=mybir.AluOpType.add)
                    t_hn = t_hb
                t_h = gsm.tile([P, CT], f32, tag="h")
                nc.scalar.activation(t_h[:], t_hn[:],
                                     mybir.ActivationFunctionType.Relu)
                m = P if b < NBLK - 1 else LASTL
                nc.sync.dma_start(d_h[b * P:b * P + m, :], t_h[0:m, :])
                if do_pool:
                    nc.tensor.matmul(t_pool[:], lhsT=t_pm[:, b:b + 1],
                                     rhs=t_h[:],
                                     start=(b == 0), stop=(b == NBLK - 1))
            if do_pool:
                t_po = constp2.tile([1, CT], f32)
                nc.vector.tensor_copy(t_po[:], t_pool[:])
                nc.sync.dma_start(d_pool[:], t_po[:])

    nc.compile()
    return nc


def _attr_array(att, CT, CE, H, scale, dtype):
    CH = CT // H
    a = np.zeros((P, CE), dtype)
    for h in range(H):
        a[:, h * (CE // H):h * (CE // H) + CH] = np.broadcast_to(
            (scale * att.reshape(H, CH)[h]).astype(dtype), (P, CH))
    return a


def _pad_xT(xT, cols):
    if xT.shape[1] == cols:
        return xT
    out = np.zeros((P, cols), np.float32)
    out[:, :xT.shape[1]] = xT
    return out


def _conv_in_maps(pre, xT, xT_locs, WA_T, WB_T, att, CT, CE, H,
                  bl, br, bo, do_pool):
    NT = (N + P - 1) // P
    DB = 4
    NMAC = (NT + DB - 1) // DB
    xTp = _pad_xT(xT, NMAC * DB * P)
    attr = _attr_array(att, CT, CE, H, 0.8, ml_dtypes.bfloat16)
    attr2 = _attr_array(att, CT, CE, H, 0.2, np.float32)
    maps = []
    for c in range(NC):
        m = {
            "xT": xTp,
            "xTl": _pad_xT(xT_locs[c], NBLK * P),
            "WA": np.ascontiguousarray(WA_T, dtype=np.float32),
            "WB": np.ascontiguousarray(WB_T, dtype=np.float32),
            "attr": attr,
            "attr2": attr2,
            "idx": pre[c]['idx_all'],
            "msel": pre[c]['msel'],
        }
        for nm, v in (("bl", bl), ("br", br), ("bo", bo)):
            if v is not None:
                m[nm] = np.ascontiguousarray(
                    np.broadcast_to(v.astype(np.float32), (P, CT)))
        if do_pool:
            m["pmask"] = pre[c]['pmask']
        maps.append(m)
    return maps


def _run(nc, maps):
    import os, time
    from concourse import bass_utils
    trace = bool(int(os.environ.get("GAT_TRACE", "0")))
    t0 = time.time()
    r = bass_utils.run_bass_kernel_spmd(nc, maps, core_ids=list(range(NC)),
                                        trace=trace)
    _cache.setdefault('run_wall', []).append(time.time() - t0)
    if getattr(r, 'exec_time_ns', None):
        _cache.setdefault('exec_ns', []).append(r.exec_time_ns)
    return r


def kernel(x, edge_index, batch, Win, b_in, Wl1, bl1, Wr1, br1, att1, bias1,
           Wl2, bl2, Wr2, br2, att2, bias2, Wout, b_out):
    x = np.asarray(x, np.float32)
    edge_index = np.asarray(edge_index)
    Win, b_in = np.asarray(Win, np.float32), np.asarray(b_in, np.float32)
    Wl1, bl1 = np.asarray(Wl1, np.float32), np.asarray(bl1, np.float32)
    Wr1, br1 = np.asarray(Wr1, np.float32), np.asarray(br1, np.float32)
    att1 = np.asarray(att1, np.float32)
    bias1 = np.asarray(bias1, np.float32)
    Wl2, bl2 = np.asarray(Wl2, np.float32), np.asarray(bl2, np.float32)
    Wr2, br2 = np.asarray(Wr2, np.float32), np.asarray(br2, np.float32)
    att2 = np.asarray(att2, np.float32)
    bias2 = np.asarray(bias2, np.float32)
    Wout, b_out = np.asarray(Wout, np.float32), np.asarray(b_out, np.float32)

    pre = _cache.get('pre')
    if pre is None or not np.array_equal(_cache.get('ei'), edge_index):
        pre = preprocess(edge_index)
        _cache['pre'] = pre
        _cache['ei'] = np.asarray(edge_index).copy()

    WA1, bA1 = Wl1 @ Win, Wl1 @ b_in + bl1
    WB1, bB1 = Wr1 @ Win, Wr1 @ b_in + br1

    xT = np.ascontiguousarray(x.T)
    xT_locs = [np.ascontiguousarray(x[c * NSH:(c + 1) * NSH].T)
               for c in range(NC)]

    if 'nc1' not in _cache:
        _cache['nc1'] = build_conv(128, 128, 2, bool(np.any(bA1)),
                                   bool(np.any(bB1)), bool(np.any(bias1)),
                                   False)
    maps1 = _conv_in_maps(pre, xT, xT_locs, WA1.T, WB1.T, att1, 128, 128, 2,
                          bA1 if np.any(bA1) else None,
                          bB1 if np.any(bB1) else None,
                          bias1 if np.any(bias1) else None, False)
    res1 = _run(_cache['nc1'], maps1)
    h1 = np.concatenate(
        [np.asarray(res1.results[c]["h_out"], np.float32)
         for c in range(NC)], 0)

    h1T = np.ascontiguousarray(h1.T)
    h1T_locs = [np.ascontiguousarray(h1[c * NSH:(c + 1) * NSH].T)
                for c in range(NC)]
    if 'nc2' not in _cache:
        _cache['nc2'] = build_conv(32, 64, 1, bool(np.any(bl2)),
                                   bool(np.any(br2)), bool(np.any(bias2)),
                                   True)
    maps2 = _conv_in_maps(pre, h1T, h1T_locs, Wl2.T, Wr2.T, att2, 32, 64, 1,
                          bl2 if np.any(bl2) else None,
                          br2 if np.any(br2) else None,
                          bias2 if np.any(bias2) else None, True)
    res2 = _run(_cache['nc2'], maps2)
    pooled = sum(np.asarray(res2.results[c]["pool_out"], np.float32)
                 for c in range(NC)).reshape(32)
    pooled = pooled / np.float32(N)
    out = pooled @ Wout.T + b_out
    return out[None, :].astype(np.float32)

